# revision 32
# baseline (speedup 1.0000x reference)
"""Multi-head attention Trainium2 Bass kernel (v6 — overlap-optimized).

Problem: B=2, T=2048, D=1024, H=16 heads, dk=64 (fp32).
  out = softmax((x@Wq.T+bq)(x@Wk.T+bk).T / 8) (x@Wv.T+bv) @ Wo.T + bo

Sharding (8 cores): data-parallel over B (2) x tensor-parallel over 4
head-groups of 4 heads.  Core (b, g) computes, for batch b and heads
[4g, 4g+4):  Q/K/V projections (column-sliced Wq/Wk/Wv), attention, and
the row-sliced Wo projection, producing a partial (2048, 1024) fp16
output.  Host sums the 4 group partials per batch and adds bo.

Design (v6):
  - bk dropped entirely (softmax exactly invariant to it); bq folded
    into the Q eviction via DVE tensor_scalar (per-partition scalar).
  - ScalarE softmax exp is the end-to-end pacer (16.8M exps/core at
    1 elem/cycle/lane @1.2GHz).  Score tiles are [128,1536] (3 PSUM
    banks, 1.5 key tiles x 2 heads) so each exp ACTIVATE amortizes its
    ~300ns overhead over 1536 columns: ~132us total.  ScalarE does exp
    (+ the K eviction copies before exp starts, + the final chunk's
    output evictions after it ends); everything else is on the DVE.
  - Startup: input DMAs split across both HWDGE queues in consumption
    order; 12 junk warmup matmuls (uninitialized-SBUF operands, no DMA
    dependency) right after the ~7us runtime prologue heat the PE HAM
    to 2.4GHz, and more junk fillers are interleaved into the K/Q
    pair-0 k-outer projection loop so the PE never idles (and never
    re-throttles) while the x stream trickles in.  K+Q pair-0 run
    k-outer simultaneously across all 8 PSUM banks in x-arrival order;
    the first softmax exp fires ~21us in.
  - PSUM budget (8 banks): ab pool 2x[128,1536] double-buffered score
    tiles + pvA/pvB [128,256] bufs=2 (2 banks total) shared by the
    pair-1/V projection accumulators, PV accumulation (per-head
    half-chunks of 256 queries), and the output projection quarters.
  - The scheduler's cost model does not know DVE RECIPROCAL is ~7x
    slower than a copy, so any tensor instruction scheduled close
    behind a reciprocal stalls the in-order tensor queue at runtime.
    Hence: normalization is chunked ([64,128] reciprocal + multiply,
    short chain links), PV double-buffering keeps the next pair's PV
    off the normalize chain, and outproj(c) is emitted TWO exp-streams
    later so its normalize dependency is long resolved by the time it
    reaches the tensor queue head.
  - Scores per key tile are an adjacent pair of K=64 matmuls on PE row
    strips 0-63/64-127 (auto tile_position) which the HW row-packs and
    runs concurrently (verified ~3ns apart in traces).
  - V_aug [V_he | ones64 | V_ho] blocks make the PV matmul emit the
    softmax denominator replicated across 64 partitions.
  - No max-subtraction in softmax (|S|/8 < ~3, fp32-exact regime).
"""

import numpy as np

D = 1024          # d_model
T = 2048          # sequence length
G = 256           # features per head-group (4 heads * 64)
DK = 64
NKT = D // 128    # 8 contraction tiles for projections
NTT = T // 128    # 16 T tiles (key tiles)
NCH = T // 512    # 4 query chunks of 512
VROW = 2 * 192    # V_aug row: 2 blocks of [V_he | ones64 | V_ho]
NU = 2 * NTT      # 32 (key-tile, head) units per (chunk, pair)
NAB = (NU + 2) // 3   # 11 score tiles per (chunk, pair)
PA_BUFS = 32      # exp tiles in flight

_CACHE = {}


def _split_multi_waits(nc):
    """walrus's TRN2 codegen rejects >1 sync-wait on datapath instruction
    structs (e.g. the fp32 self-loading matmul's LDWEIGHTS part, tensor-
    scalar).  Hoist every wait of a multi-wait datapath instruction onto
    single-wait NoOps just before it on the same engine queue - semantically
    identical (engine executes in order) and each NoOp carries one wait."""
    import concourse.mybir as mybir

    keep = ("InstEventSemaphore", "InstUnconditionalBranch",
            "InstCall", "InstBranchHint", "InstHalt", "InstNoOp",
            "InstAllEngineBarrier", "InstCompareAndBranch")
    nid = [0]
    for f in nc.m.functions:
        for bb in f.blocks:
            new = []
            for ins in bb.instructions:
                si = ins.sync_info
                waits = list(si.on_wait) if si and si.on_wait else []
                if len(waits) >= 2 and type(ins).__name__ not in keep:
                    for w in waits:
                        nid[0] += 1
                        nop = mybir.InstNoOp(name=f"{ins.name}-wsplit{nid[0]}",
                                             ins=[], outs=[])
                        nop.engine = ins.engine
                        nop.sync_info = mybir.SyncInfo(on_wait=[w], on_update=[])
                        new.append(nop)
                    ins.sync_info = mybir.SyncInfo(
                        on_wait=[], on_update=list(si.on_update or []))
                new.append(ins)
            bb.instructions = new


def _build(split_waits=True, compute_dt="float16"):
    import concourse.bass as bass
    import concourse.mybir as mybir
    import concourse.tile as tile

    f32 = mybir.dt.float32
    fc = getattr(mybir.dt, compute_dt)
    nc = bass.Bass()

    xT = nc.dram_tensor("xT", [D, T], fc, kind="ExternalInput")
    wqT = nc.dram_tensor("wqT", [D, G], fc, kind="ExternalInput")
    wkT = nc.dram_tensor("wkT", [D, G], fc, kind="ExternalInput")
    wvT = nc.dram_tensor("wvT", [D, G], fc, kind="ExternalInput")
    woT = nc.dram_tensor("woT", [G, D], fc, kind="ExternalInput")
    bqT = nc.dram_tensor("bqT", [G, 1], f32, kind="ExternalInput")
    bv = nc.dram_tensor("bv", [1, G], fc, kind="ExternalInput")
    out = nc.dram_tensor("out", [T, D], fc, kind="ExternalOutput")

    EXP = mybir.ActivationFunctionType.Exp
    COPY = mybir.ActivationFunctionType.Copy

    with tile.TileContext(nc) as tc:
        with tc.tile_pool(name="sb", bufs=1) as sb, \
             tc.tile_pool(name="dyn", bufs=2) as dyn, \
             tc.tile_pool(name="ps_ab", bufs=1, space="PSUM") as ps_ab, \
             tc.tile_pool(name="ps_w", bufs=1, space="PSUM") as ps_w:
            # PSUM (8 banks, bank-granular): ab3 [128,1536] + ab2 [128,1024]
            # alternating score tiles (5 banks, behaves like one
            # double-buffered tag), pvA [128,512] bufs=2 + pvB bufs=1.

            def ab_tile(i, name):
                tag = ("ab3", "ab2")[i % 2]
                shape = [128, 1536] if tag == "ab3" else [128, 1024]
                return ps_ab.tile(shape, f32, tag=tag, name=name), shape[1] // 512

            def pv_tile(h, name):
                return ps_w.tile([128, 512], f32, tag=("pvA", "pvB")[h],
                                 bufs=1, name=name)

            def op_tile(name):
                return ps_w.tile([128, 512], f32, tag="op", bufs=1, name=name)

            # ---- constants + exp table preload ----
            ones_row = sb.tile([1, G], fc, tag="ones", name="ones_row")
            nc.vector.memset(ones_row, 1.0)
            dummy = sb.tile([1, 1], fc, tag="dummy", name="dummy")
            nc.scalar.activation(out=dummy, in_=ones_row[0:1, 0:1], func=EXP)

            # ---- input DMAs, split across both HWDGE queues ----
            xt = [None] * NKT
            wk_sb, wq_sb, wv_sb = [], [], []
            # sync queue: wk+wq (small, needed first), then x0..x2
            for k in range(NKT):
                t = sb.tile([128, G], fc, tag=f"wk{k}", name=f"wk{k}")
                nc.sync.dma_start(out=t, in_=wkT[k * 128:(k + 1) * 128, :])
                wk_sb.append(t)
                t = sb.tile([128, G], fc, tag=f"wq{k}", name=f"wq{k}")
                nc.sync.dma_start(out=t, in_=wqT[k * 128:(k + 1) * 128, :])
                wq_sb.append(t)
            for k in range(3):
                t = sb.tile([128, T], fc, tag=f"xt{k}", name=f"xt{k}")
                nc.sync.dma_start(out=t, in_=xT[k * 128:(k + 1) * 128, :])
                xt[k] = t
            # scalar queue: x4..x6, then biases, wv, wo
            for k in (4, 5, 6):
                t = sb.tile([128, T], fc, tag=f"xt{k}", name=f"xt{k}")
                nc.scalar.dma_start(out=t, in_=xT[k * 128:(k + 1) * 128, :])
                xt[k] = t
            # gpsimd SWDGE queue: x3, x7 (third parallel DMA path)
            for k in (3, 7):
                t = sb.tile([128, T], fc, tag=f"xt{k}", name=f"xt{k}")
                nc.gpsimd.dma_start(out=t, in_=xT[k * 128:(k + 1) * 128, :])
                xt[k] = t
            bq_sb = sb.tile([128, 2], f32, tag="bq", name="bq_sb")
            nc.scalar.dma_start(out=bq_sb[:, 0:1], in_=bqT[0:128, :])
            nc.scalar.dma_start(out=bq_sb[:, 1:2], in_=bqT[128:256, :])
            bv_sb = sb.tile([1, G], fc, tag="bv", name="bv_sb")
            nc.scalar.dma_start(out=bv_sb, in_=bv[:, :])
            for k in range(NKT):
                t = sb.tile([128, G], fc, tag=f"wv{k}", name=f"wv{k}")
                nc.scalar.dma_start(out=t, in_=wvT[k * 128:(k + 1) * 128, :])
                wv_sb.append(t)
            wo_sb = []
            for p2 in range(2):
                t = sb.tile([128, D], fc, tag=f"wo{p2}", name=f"wo{p2}")
                nc.scalar.dma_start(out=t, in_=woT[p2 * 128:(p2 + 1) * 128, :])
                wo_sb.append(t)

            # V_aug storage: per T-tile 2 blocks of [V_he|ones64|V_ho]
            va = sb.tile([128, NTT * VROW], fc, tag="va", name="va")
            va_view = va.rearrange("p (t b x) -> p t b x", t=NTT, b=2)
            for b2 in range(2):
                nc.vector.memset(va_view[:, :, b2, 64:128], 1.0)

            qt = [sb.tile([128, T], fc, tag=f"qt{p}", name=f"qt{p}")
                  for p in range(2)]
            kt = [sb.tile([128, T], fc, tag=f"kt{p}", name=f"kt{p}")
                  for p in range(2)]

            # ---- HAM warmup: junk matmuls on an uninitialized SBUF tile
            # (no DMA dependency) heat the PE right after the prologue.
            junk = sb.tile([128, 512], fc, tag="junk", name="junk")
            nc.vector.memset(junk, 1.0)
            warm, _ = ab_tile(1, "warm")   # ab2 bank; released at phase-A end

            def filler(n):
                for _ in range(n):
                    nc.tensor.matmul(out=warm[:, 0:512], lhsT=junk[:, 0:128],
                                     rhs=junk[:, :], start=True, stop=True)

            filler(12)

            # ---- K pair-0 (all chunks) + Q pair-0 (chunks 0,1): k-outer in
            # x-tile arrival order, junk fillers interleaved so the PE never
            # re-throttles while the x stream trickles in.  scores(0,0) only
            # needs Q chunk 0, so Q0 c2/c3 are computed later as stream
            # filler.
            kq_a, _ = ab_tile(2, "kq_a")    # ab3: K0 c0,c1,c2
            pv_k3 = op_tile("kq_k3")        # op bank: K0 c3
            pv_q0 = pv_tile(0, "kq_q0")     # pvA: Q0 c0
            pv_q1 = pv_tile(1, "kq_q1")     # pvB: Q0 c1
            kacc = [kq_a[:, 0:512], kq_a[:, 512:1024], kq_a[:, 1024:1536],
                    pv_k3]
            qacc = [pv_q0, pv_q1]
            korder = [4, 0, 5, 1, 6, 2, 3, 7]   # x-tile arrival order
            for i, k in enumerate(korder):
                st, sp = (i == 0), (i == len(korder) - 1)
                for c in range(NCH):
                    nc.tensor.matmul(out=kacc[c], lhsT=wk_sb[k][:, 0:128],
                                     rhs=xt[k][:, c * 512:(c + 1) * 512],
                                     start=st, stop=sp)
                for c in range(2):
                    nc.tensor.matmul(out=qacc[c], lhsT=wq_sb[k][:, 0:128],
                                     rhs=xt[k][:, c * 512:(c + 1) * 512],
                                     start=st, stop=sp)
                filler(2)
            # evictions: kt on ScalarE (idle until exp starts), qt on DVE
            for c in range(NCH):
                nc.scalar.activation(out=kt[0][:, c * 512:(c + 1) * 512],
                                     in_=kacc[c], func=COPY)
            for c in range(2):
                nc.vector.tensor_scalar_add(qt[0][:, c * 512:(c + 1) * 512],
                                            qacc[c], bq_sb[:, 0:1])

            def q0_rest():
                for cc in (2, 3):
                    acc = pv_tile(cc - 2, f"q0r{cc}")
                    for k in range(NKT):
                        nc.tensor.matmul(
                            out=acc, lhsT=wq_sb[k][:, 0:128],
                            rhs=xt[k][:, cc * 512:(cc + 1) * 512],
                            start=(k == 0), stop=(k == NKT - 1))
                    nc.vector.tensor_scalar_add(
                        qt[0][:, cc * 512:(cc + 1) * 512], acc, bq_sb[:, 0:1])

            # ---- K/Q pair-1 (k-inner, 2 chunks at a time; x resident) ----
            def kq_proj(p2, w_sb, dst, bias):
                for cc in (0, 2):
                    accs = [pv_tile(h, f"kq{p2}_{cc}_{h}") for h in range(2)]
                    for k in range(NKT):
                        for j in range(2):
                            c = cc + j
                            nc.tensor.matmul(
                                out=accs[j],
                                lhsT=w_sb[k][:, p2 * 128:(p2 + 1) * 128],
                                rhs=xt[k][:, c * 512:(c + 1) * 512],
                                start=(k == 0), stop=(k == NKT - 1))
                    for j in range(2):
                        c = cc + j
                        if bias is None:
                            nc.vector.tensor_copy(
                                out=dst[:, c * 512:(c + 1) * 512], in_=accs[j])
                        else:
                            nc.vector.tensor_scalar_add(
                                dst[:, c * 512:(c + 1) * 512], accs[j], bias)

            # ---- V projection (k-inner through pvA/pvB) ----
            def v_proj():
                for tt in range(NTT):
                    ps = pv_tile(tt % 2, f"v{tt}")
                    psv = ps[:, 0:G]
                    nc.tensor.matmul(out=psv, lhsT=ones_row[0:1, 0:128],
                                     rhs=bv_sb[0:1, :], start=True, stop=False)
                    for k in range(NKT):
                        nc.tensor.matmul(
                            out=psv,
                            lhsT=xt[k][:, tt * 128:(tt + 1) * 128],
                            rhs=wv_sb[k][:, :],
                            start=False, stop=(k == NKT - 1))
                    pr = psv.rearrange("p (b h x) -> p h b x", b=2, h=2)
                    nc.vector.tensor_copy(out=va_view[:, tt, :, 0:64],
                                          in_=pr[:, 0, :, :])
                    nc.vector.tensor_copy(out=va_view[:, tt, :, 128:192],
                                          in_=pr[:, 1, :, :])

            # ---- attention stages ----
            # Units: u in [0,32) -> key tile u//2, head u%2.  Score tiles
            # alternate ab3 (3 units) / ab2 (2 units); exp per tile.
            ab_idx = [3]   # global alternation counter (warm=1, kq_a=2)

            def attn_scores(c, pair):
                pa_locs = []
                u = 0
                while u < NU:
                    ab, slots = ab_tile(ab_idx[0], f"s{c}_{pair}_{u}")
                    ab_idx[0] += 1
                    nslots = min(slots, NU - u)
                    width = 512 * nslots
                    for s in range(nslots):
                        g, h = divmod(u, 2)
                        nc.tensor.matmul(
                            out=ab[:, s * 512:(s + 1) * 512],
                            lhsT=kt[pair][h * 64:(h + 1) * 64,
                                          g * 128:(g + 1) * 128],
                            rhs=qt[pair][h * 64:(h + 1) * 64,
                                         c * 512:(c + 1) * 512],
                            start=True, stop=True)
                        u += 1
                    pa = dyn.tile([128, 1536], fc, tag="pa", bufs=PA_BUFS,
                                  name=f"p{c}_{pair}_{u}")
                    nc.scalar.activation(out=pa[:, 0:width], in_=ab[:, 0:width],
                                         func=EXP, scale=0.125)
                    for s in range(nslots):
                        pa_locs.append((pa, s * 512))
                return pa_locs

            def attn_pv(c, pair, pa_locs):
                """PV accumulation; PSUM is evicted to SBUF immediately (two
                cheap copies) so the banks free in ~1.4us instead of being
                held through the ~8us normalize chain; normalization then
                runs from SBUF entirely off the PSUM dependency chain."""
                pv = [pv_tile(h, f"pv{h}_{c}_{pair}") for h in range(2)]
                for g in range(NTT):
                    off = g * VROW + pair * 192
                    p0, o0 = pa_locs[2 * g]
                    p1, o1 = pa_locs[2 * g + 1]
                    nc.tensor.matmul(
                        out=pv[0], lhsT=va[:, off:off + 128],
                        rhs=p0[:, o0:o0 + 512],
                        start=(g == 0), stop=(g == NTT - 1))
                    nc.tensor.matmul(
                        out=pv[1], lhsT=va[:, off + 64:off + 192],
                        rhs=p1[:, o1:o1 + 512],
                        start=(g == 0), stop=(g == NTT - 1))
                # Eviction does the partition alignment (PSUM reads can land
                # on any SBUF base partition): ovs = [O_h0 ; O_h1], dns =
                # [denom_h0 ; denom_h1] -> one reciprocal + one multiply for
                # both heads.
                ovs = dyn.tile([128, 512], f32, tag="ovs", bufs=2,
                               name=f"ovs_{c}_{pair}")
                dns = dyn.tile([128, 512], f32, tag="dns", bufs=2,
                               name=f"dns_{c}_{pair}")
                nc.vector.tensor_copy(out=ovs[0:64, :], in_=pv[0][0:64, :])
                nc.vector.tensor_copy(out=dns[0:64, :], in_=pv[0][64:128, :])
                nc.vector.tensor_copy(out=ovs[64:128, :], in_=pv[1][64:128, :])
                nc.vector.tensor_copy(out=dns[64:128, :], in_=pv[1][0:64, :])
                on = dyn.tile([128, 512], fc, tag=f"on{pair}", bufs=3,
                              name=f"on{pair}_{c}")
                rc = dyn.tile([128, 512], f32, tag="rc", bufs=2,
                              name=f"rc{c}_{pair}")
                nc.vector.reciprocal(out=rc, in_=dns)
                nc.vector.tensor_mul(on, ovs, rc)
                return on

            def outproj(c, ons):
                for mt in range(4):
                    osb = dyn.tile([128, 1024], fc, tag="osb", bufs=2,
                                   name=f"osb{c}_{mt}")
                    for n2 in range(2):
                        ps = op_tile(f"op{c}_{mt}_{n2}")
                        for pair in range(2):
                            nc.tensor.matmul(
                                out=ps,
                                lhsT=ons[pair][:, mt * 128:(mt + 1) * 128],
                                rhs=wo_sb[pair][:, n2 * 512:(n2 + 1) * 512],
                                start=(pair == 0), stop=(pair == 1))
                        nc.vector.tensor_copy(
                            out=osb[:, n2 * 512:(n2 + 1) * 512], in_=ps)
                    nc.sync.dma_start(
                        out=out[c * 512 + mt * 128:c * 512 + (mt + 1) * 128, :],
                        in_=osb)

            def tail_finish(c, pa_locs, on0):
                """Last (chunk, pair): PV + latency-optimized normalize +
                output projection.  Only the denominators are copied out of
                PSUM (muls read PSUM directly - mixed-space tensor_tensor is
                allowed); the reciprocal is split in halves with the output
                projection interleaved; outproj accumulates in the freed
                score banks (fully pipelined) and evicts on the idle
                ScalarE; junk fillers keep the PE warm through the DVE
                normalize window."""
                pv = [pv_tile(h, f"pvT{h}") for h in range(2)]
                for g in range(NTT):
                    off = g * VROW + 192
                    p0, o0 = pa_locs[2 * g]
                    p1, o1 = pa_locs[2 * g + 1]
                    nc.tensor.matmul(
                        out=pv[0], lhsT=va[:, off:off + 128],
                        rhs=p0[:, o0:o0 + 512],
                        start=(g == 0), stop=(g == NTT - 1))
                    nc.tensor.matmul(
                        out=pv[1], lhsT=va[:, off + 64:off + 192],
                        rhs=p1[:, o1:o1 + 512],
                        start=(g == 0), stop=(g == NTT - 1))
                wt, _ = ab_tile(ab_idx[0], "wtail")
                ab_idx[0] += 1
                for _ in range(10):
                    nc.tensor.matmul(out=wt[:, 0:512], lhsT=junk[:, 0:128],
                                     rhs=junk[:, 0:512], start=True, stop=True)
                dns = dyn.tile([128, 512], f32, tag="dns", bufs=2,
                               name="dnsT")
                nc.vector.tensor_copy(out=dns[0:64, :], in_=pv[0][64:128, :])
                nc.vector.tensor_copy(out=dns[64:128, :], in_=pv[1][0:64, :])
                on1 = dyn.tile([128, 512], fc, tag="on1", bufs=3, name="onT")
                rc = dyn.tile([128, 512], f32, tag="rc", bufs=2, name="rcT")
                ons = (on0, on1)
                slots = []   # (tile, offset) outproj accumulators in ab banks
                for half in range(2):
                    hs = slice(half * 256, (half + 1) * 256)
                    nc.vector.reciprocal(out=rc[:, hs], in_=dns[:, hs])
                    nc.vector.tensor_mul(on1[0:64, hs], pv[0][0:64, hs],
                                         rc[0:64, hs])
                    nc.vector.tensor_mul(on1[64:128, hs], pv[1][64:128, hs],
                                         rc[64:128, hs])
                    for mt in (2 * half, 2 * half + 1):
                        osb = dyn.tile([128, 1024], fc, tag="osb", bufs=2,
                                       name=f"osbT_{mt}")
                        for n2 in range(2):
                            if not slots:
                                t, ns = ab_tile(ab_idx[0], f"opT{mt}{n2}")
                                ab_idx[0] += 1
                                slots = [t[:, s * 512:(s + 1) * 512]
                                         for s in range(ns)]
                            ps = slots.pop(0)
                            for pair in range(2):
                                nc.tensor.matmul(
                                    out=ps,
                                    lhsT=ons[pair][:, mt * 128:(mt + 1) * 128],
                                    rhs=wo_sb[pair][:, n2 * 512:(n2 + 1) * 512],
                                    start=(pair == 0), stop=(pair == 1))
                            nc.scalar.activation(
                                out=osb[:, n2 * 512:(n2 + 1) * 512], in_=ps,
                                func=COPY)
                        nc.sync.dma_start(
                            out=out[c * 512 + mt * 128:
                                    c * 512 + (mt + 1) * 128, :],
                            in_=osb)

            # ---- schedule ----
            # Score streams lead in priority (ScalarE never starves); each
            # pair's PV+normalize follows its own stream; outproj trails by
            # TWO streams so its (mis-cost-modeled) normalize dependency
            # can never stall the tensor queue.
            sc00 = attn_scores(0, 0)       # exp starts here (~21us)
            kq_proj(1, wk_sb, kt[1], None)          # under sc00's exp stream
            kq_proj(1, wq_sb, qt[1], bq_sb[:, 1:2])
            sc01 = attn_scores(0, 1)
            v_proj()                                # under sc01's exp stream
            sc10 = attn_scores(1, 0)
            on00 = attn_pv(0, 0, sc00)              # bursts after V is done
            on01 = attn_pv(0, 1, sc01)
            on10 = attn_pv(1, 0, sc10)
            sc11 = attn_scores(1, 1)
            on11 = attn_pv(1, 1, sc11)
            q0_rest()
            sc20 = attn_scores(2, 0)
            on20 = attn_pv(2, 0, sc20)
            outproj(0, (on00, on01))
            sc21 = attn_scores(2, 1)
            on21 = attn_pv(2, 1, sc21)
            outproj(1, (on10, on11))
            sc30 = attn_scores(3, 0)
            on30 = attn_pv(3, 0, sc30)
            outproj(2, (on20, on21))
            sc31 = attn_scores(3, 1)
            tail_finish(3, sc31, on30)

    if split_waits:
        _split_multi_waits(nc)
    return nc


COMPUTE_DT = "float16"   # matmul operand dtype; PSUM accumulation is fp32


def _get_nc(split_waits=True, compute_dt=COMPUTE_DT):
    key = ("nc", split_waits, compute_dt)
    if key not in _CACHE:
        _CACHE[key] = _build(split_waits, compute_dt)
    return _CACHE[key]


def _np_dt():
    return {"float16": np.float16, "bfloat16": None,
            "float32": np.float32}[COMPUTE_DT]


def make_in_maps(x, Wq, bq, Wk, bk, Wv, bv, Wo):
    # bk is intentionally unused: softmax is exactly invariant to it.
    dt = _np_dt()
    in_maps = []
    for core in range(8):
        b, g = divmod(core, 4)
        gs = slice(g * G, (g + 1) * G)
        in_maps.append({
            "xT": np.ascontiguousarray(x[b].T).astype(dt),
            "wqT": np.ascontiguousarray(Wq[gs, :].T).astype(dt),
            "wkT": np.ascontiguousarray(Wk[gs, :].T).astype(dt),
            "wvT": np.ascontiguousarray(Wv[gs, :].T).astype(dt),
            "woT": np.ascontiguousarray(Wo[:, gs].T).astype(dt),
            "bqT": np.ascontiguousarray(bq[gs].reshape(G, 1)).astype(np.float32),
            "bv": np.ascontiguousarray(bv[gs].reshape(1, G)).astype(dt),
        })
    return in_maps


def kernel(x, Wq, bq, Wk, bk, Wv, bv, Wo, bo):
    from concourse.bass_utils import run_bass_kernel_spmd

    x = np.asarray(x, dtype=np.float32)
    Wq = np.asarray(Wq, dtype=np.float32)
    Wk = np.asarray(Wk, dtype=np.float32)
    Wv = np.asarray(Wv, dtype=np.float32)
    Wo = np.asarray(Wo, dtype=np.float32)
    bq = np.asarray(bq, dtype=np.float32)
    bv = np.asarray(bv, dtype=np.float32)
    bo = np.asarray(bo, dtype=np.float32)

    nc = _get_nc()
    in_maps = make_in_maps(x, Wq, bq, Wk, None, Wv, bv, Wo)

    res = run_bass_kernel_spmd(nc, in_maps, core_ids=list(range(8)))
    outp = np.tile(bo[None, None, :], (2, T, 1)).astype(np.float32)
    for core in range(8):
        b = core // 4
        outp[b] += res.results[core]["out"].astype(np.float32)
    return outp


# revision 34
# speedup vs baseline: 1.0043x; 1.0043x over previous
"""Multi-head attention Trainium2 Bass kernel (v6 — overlap-optimized).

Problem: B=2, T=2048, D=1024, H=16 heads, dk=64 (fp32).
  out = softmax((x@Wq.T+bq)(x@Wk.T+bk).T / 8) (x@Wv.T+bv) @ Wo.T + bo

Sharding (8 cores): data-parallel over B (2) x tensor-parallel over 4
head-groups of 4 heads.  Core (b, g) computes, for batch b and heads
[4g, 4g+4):  Q/K/V projections (column-sliced Wq/Wk/Wv), attention, and
the row-sliced Wo projection, producing a partial (2048, 1024) fp16
output.  Host sums the 4 group partials per batch and adds bo.

Design (v6):
  - bk dropped entirely (softmax exactly invariant to it); bq folded
    into the Q eviction via DVE tensor_scalar (per-partition scalar).
  - ScalarE softmax exp is the end-to-end pacer (16.8M exps/core at
    1 elem/cycle/lane @1.2GHz).  Score tiles are [128,1536] (3 PSUM
    banks, 1.5 key tiles x 2 heads) so each exp ACTIVATE amortizes its
    ~300ns overhead over 1536 columns: ~132us total.  ScalarE does exp
    (+ the K eviction copies before exp starts, + the final chunk's
    output evictions after it ends); everything else is on the DVE.
  - Startup: input DMAs split across both HWDGE queues in consumption
    order; 12 junk warmup matmuls (uninitialized-SBUF operands, no DMA
    dependency) right after the ~7us runtime prologue heat the PE HAM
    to 2.4GHz, and more junk fillers are interleaved into the K/Q
    pair-0 k-outer projection loop so the PE never idles (and never
    re-throttles) while the x stream trickles in.  K+Q pair-0 run
    k-outer simultaneously across all 8 PSUM banks in x-arrival order;
    the first softmax exp fires ~21us in.
  - PSUM budget (8 banks): ab pool 2x[128,1536] double-buffered score
    tiles + pvA/pvB [128,256] bufs=2 (2 banks total) shared by the
    pair-1/V projection accumulators, PV accumulation (per-head
    half-chunks of 256 queries), and the output projection quarters.
  - The scheduler's cost model does not know DVE RECIPROCAL is ~7x
    slower than a copy, so any tensor instruction scheduled close
    behind a reciprocal stalls the in-order tensor queue at runtime.
    Hence: normalization is chunked ([64,128] reciprocal + multiply,
    short chain links), PV double-buffering keeps the next pair's PV
    off the normalize chain, and outproj(c) is emitted TWO exp-streams
    later so its normalize dependency is long resolved by the time it
    reaches the tensor queue head.
  - Scores per key tile are an adjacent pair of K=64 matmuls on PE row
    strips 0-63/64-127 (auto tile_position) which the HW row-packs and
    runs concurrently (verified ~3ns apart in traces).
  - V_aug [V_he | ones64 | V_ho] blocks make the PV matmul emit the
    softmax denominator replicated across 64 partitions.
  - No max-subtraction in softmax (|S|/8 < ~3, fp32-exact regime).
"""

import numpy as np

D = 1024          # d_model
T = 2048          # sequence length
G = 256           # features per head-group (4 heads * 64)
DK = 64
NKT = D // 128    # 8 contraction tiles for projections
NTT = T // 128    # 16 T tiles (key tiles)
NCH = T // 512    # 4 query chunks of 512
VROW = 2 * 192    # V_aug row: 2 blocks of [V_he | ones64 | V_ho]
NU = 2 * NTT      # 32 (key-tile, head) units per (chunk, pair)
NAB = (NU + 2) // 3   # 11 score tiles per (chunk, pair)
PA_BUFS = 32      # exp tiles in flight

_CACHE = {}


def _split_multi_waits(nc):
    """walrus's TRN2 codegen rejects >1 sync-wait on datapath instruction
    structs (e.g. the fp32 self-loading matmul's LDWEIGHTS part, tensor-
    scalar).  Hoist every wait of a multi-wait datapath instruction onto
    single-wait NoOps just before it on the same engine queue - semantically
    identical (engine executes in order) and each NoOp carries one wait."""
    import concourse.mybir as mybir

    keep = ("InstEventSemaphore", "InstUnconditionalBranch",
            "InstCall", "InstBranchHint", "InstHalt", "InstNoOp",
            "InstAllEngineBarrier", "InstCompareAndBranch")
    nid = [0]
    for f in nc.m.functions:
        for bb in f.blocks:
            new = []
            for ins in bb.instructions:
                si = ins.sync_info
                waits = list(si.on_wait) if si and si.on_wait else []
                if len(waits) >= 2 and type(ins).__name__ not in keep:
                    for w in waits:
                        nid[0] += 1
                        nop = mybir.InstNoOp(name=f"{ins.name}-wsplit{nid[0]}",
                                             ins=[], outs=[])
                        nop.engine = ins.engine
                        nop.sync_info = mybir.SyncInfo(on_wait=[w], on_update=[])
                        new.append(nop)
                    ins.sync_info = mybir.SyncInfo(
                        on_wait=[], on_update=list(si.on_update or []))
                new.append(ins)
            bb.instructions = new


def _build(split_waits=True, compute_dt="float16"):
    import concourse.bass as bass
    import concourse.mybir as mybir
    import concourse.tile as tile

    f32 = mybir.dt.float32
    fc = getattr(mybir.dt, compute_dt)
    nc = bass.Bass()

    xT = nc.dram_tensor("xT", [D, T], fc, kind="ExternalInput")
    wqT = nc.dram_tensor("wqT", [D, G], fc, kind="ExternalInput")
    wkT = nc.dram_tensor("wkT", [D, G], fc, kind="ExternalInput")
    wvT = nc.dram_tensor("wvT", [D, G], fc, kind="ExternalInput")
    woT = nc.dram_tensor("woT", [G, D], fc, kind="ExternalInput")
    bqT = nc.dram_tensor("bqT", [G, 1], f32, kind="ExternalInput")
    bv = nc.dram_tensor("bv", [1, G], fc, kind="ExternalInput")
    out = nc.dram_tensor("out", [T, D], fc, kind="ExternalOutput")

    EXP = mybir.ActivationFunctionType.Exp
    COPY = mybir.ActivationFunctionType.Copy

    with tile.TileContext(nc) as tc:
        with tc.tile_pool(name="sb", bufs=1) as sb, \
             tc.tile_pool(name="dyn", bufs=2) as dyn, \
             tc.tile_pool(name="ps_ab", bufs=1, space="PSUM") as ps_ab, \
             tc.tile_pool(name="ps_w", bufs=1, space="PSUM") as ps_w:
            # PSUM (8 banks, bank-granular): ab3 [128,1536] + ab2 [128,1024]
            # alternating score tiles (5 banks, behaves like one
            # double-buffered tag), pvA [128,512] bufs=2 + pvB bufs=1.

            def ab_tile(i, name):
                tag = ("ab3", "ab2")[i % 2]
                shape = [128, 1536] if tag == "ab3" else [128, 1024]
                return ps_ab.tile(shape, f32, tag=tag, name=name), shape[1] // 512

            def pv_tile(h, name):
                return ps_w.tile([128, 512], f32, tag=("pvA", "pvB")[h],
                                 bufs=1, name=name)

            def op_tile(name):
                return ps_w.tile([128, 512], f32, tag="op", bufs=1, name=name)

            # ---- constants + exp table preload ----
            ones_row = sb.tile([1, G], fc, tag="ones", name="ones_row")
            nc.vector.memset(ones_row, 1.0)
            dummy = sb.tile([1, 1], fc, tag="dummy", name="dummy")
            nc.scalar.activation(out=dummy, in_=ones_row[0:1, 0:1], func=EXP)

            # ---- input DMAs, split across both HWDGE queues ----
            xt = [None] * NKT
            wk_sb, wq_sb, wv_sb = [], [], []
            # sync queue: wk+wq (small, needed first), then x0..x2
            for k in range(NKT):
                t = sb.tile([128, G], fc, tag=f"wk{k}", name=f"wk{k}")
                nc.sync.dma_start(out=t, in_=wkT[k * 128:(k + 1) * 128, :])
                wk_sb.append(t)
                t = sb.tile([128, G], fc, tag=f"wq{k}", name=f"wq{k}")
                nc.sync.dma_start(out=t, in_=wqT[k * 128:(k + 1) * 128, :])
                wq_sb.append(t)
            for k in range(3):
                t = sb.tile([128, T], fc, tag=f"xt{k}", name=f"xt{k}")
                nc.sync.dma_start(out=t, in_=xT[k * 128:(k + 1) * 128, :])
                xt[k] = t
            # scalar queue: x4..x6, then biases, wv, wo
            for k in (4, 5, 6):
                t = sb.tile([128, T], fc, tag=f"xt{k}", name=f"xt{k}")
                nc.scalar.dma_start(out=t, in_=xT[k * 128:(k + 1) * 128, :])
                xt[k] = t
            # gpsimd SWDGE queue: x3, x7 (third parallel DMA path)
            for k in (3, 7):
                t = sb.tile([128, T], fc, tag=f"xt{k}", name=f"xt{k}")
                nc.gpsimd.dma_start(out=t, in_=xT[k * 128:(k + 1) * 128, :])
                xt[k] = t
            bq_sb = sb.tile([128, 2], f32, tag="bq", name="bq_sb")
            nc.scalar.dma_start(out=bq_sb[:, 0:1], in_=bqT[0:128, :])
            nc.scalar.dma_start(out=bq_sb[:, 1:2], in_=bqT[128:256, :])
            bv_sb = sb.tile([1, G], fc, tag="bv", name="bv_sb")
            nc.scalar.dma_start(out=bv_sb, in_=bv[:, :])
            for k in range(NKT):
                t = sb.tile([128, G], fc, tag=f"wv{k}", name=f"wv{k}")
                nc.scalar.dma_start(out=t, in_=wvT[k * 128:(k + 1) * 128, :])
                wv_sb.append(t)
            wo_sb = []
            for p2 in range(2):
                t = sb.tile([128, D], fc, tag=f"wo{p2}", name=f"wo{p2}")
                nc.scalar.dma_start(out=t, in_=woT[p2 * 128:(p2 + 1) * 128, :])
                wo_sb.append(t)

            # V_aug storage: per T-tile 2 blocks of [V_he|ones64|V_ho]
            va = sb.tile([128, NTT * VROW], fc, tag="va", name="va")
            va_view = va.rearrange("p (t b x) -> p t b x", t=NTT, b=2)
            for b2 in range(2):
                nc.vector.memset(va_view[:, :, b2, 64:128], 1.0)

            qt = [sb.tile([128, T], fc, tag=f"qt{p}", name=f"qt{p}")
                  for p in range(2)]
            kt = [sb.tile([128, T], fc, tag=f"kt{p}", name=f"kt{p}")
                  for p in range(2)]

            # ---- HAM warmup: junk matmuls on an uninitialized SBUF tile
            # (no DMA dependency) heat the PE right after the prologue.
            junk = sb.tile([128, 512], fc, tag="junk", name="junk")
            nc.vector.memset(junk, 1.0)
            warm, _ = ab_tile(1, "warm")   # ab2 bank; released at phase-A end

            def filler(n):
                for _ in range(n):
                    nc.tensor.matmul(out=warm[:, 0:512], lhsT=junk[:, 0:128],
                                     rhs=junk[:, :], start=True, stop=True)

            filler(12)

            # ---- K pair-0 (all chunks) + Q pair-0 (chunks 0,1): k-outer in
            # x-tile arrival order, junk fillers interleaved so the PE never
            # re-throttles while the x stream trickles in.  scores(0,0) only
            # needs Q chunk 0, so Q0 c2/c3 are computed later as stream
            # filler.
            kq_a, _ = ab_tile(2, "kq_a")    # ab3: K0 c0,c1,c2
            pv_k3 = op_tile("kq_k3")        # op bank: K0 c3
            pv_q0 = pv_tile(0, "kq_q0")     # pvA: Q0 c0
            pv_q1 = pv_tile(1, "kq_q1")     # pvB: Q0 c1
            kacc = [kq_a[:, 0:512], kq_a[:, 512:1024], kq_a[:, 1024:1536],
                    pv_k3]
            qacc = [pv_q0, pv_q1]
            korder = [4, 0, 5, 1, 6, 2, 3, 7]   # x-tile arrival order
            for i, k in enumerate(korder):
                st, sp = (i == 0), (i == len(korder) - 1)
                for c in range(NCH):
                    nc.tensor.matmul(out=kacc[c], lhsT=wk_sb[k][:, 0:128],
                                     rhs=xt[k][:, c * 512:(c + 1) * 512],
                                     start=st, stop=sp)
                for c in range(2):
                    nc.tensor.matmul(out=qacc[c], lhsT=wq_sb[k][:, 0:128],
                                     rhs=xt[k][:, c * 512:(c + 1) * 512],
                                     start=st, stop=sp)
                filler(2)
            # evictions: kt on ScalarE (idle until exp starts), qt on DVE
            for c in range(NCH):
                nc.scalar.activation(out=kt[0][:, c * 512:(c + 1) * 512],
                                     in_=kacc[c], func=COPY)
            for c in range(2):
                nc.vector.tensor_scalar_add(qt[0][:, c * 512:(c + 1) * 512],
                                            qacc[c], bq_sb[:, 0:1])

            def q0_rest():
                for cc in (2, 3):
                    acc = pv_tile(cc - 2, f"q0r{cc}")
                    for k in range(NKT):
                        nc.tensor.matmul(
                            out=acc, lhsT=wq_sb[k][:, 0:128],
                            rhs=xt[k][:, cc * 512:(cc + 1) * 512],
                            start=(k == 0), stop=(k == NKT - 1))
                    nc.vector.tensor_scalar_add(
                        qt[0][:, cc * 512:(cc + 1) * 512], acc, bq_sb[:, 0:1])

            # ---- K/Q pair-1 (k-inner, 2 chunks at a time; x resident) ----
            def kq_proj(p2, w_sb, dst, bias):
                for cc in (0, 2):
                    accs = [pv_tile(h, f"kq{p2}_{cc}_{h}") for h in range(2)]
                    for k in range(NKT):
                        for j in range(2):
                            c = cc + j
                            nc.tensor.matmul(
                                out=accs[j],
                                lhsT=w_sb[k][:, p2 * 128:(p2 + 1) * 128],
                                rhs=xt[k][:, c * 512:(c + 1) * 512],
                                start=(k == 0), stop=(k == NKT - 1))
                    for j in range(2):
                        c = cc + j
                        if bias is None:
                            nc.vector.tensor_copy(
                                out=dst[:, c * 512:(c + 1) * 512], in_=accs[j])
                        else:
                            nc.vector.tensor_scalar_add(
                                dst[:, c * 512:(c + 1) * 512], accs[j], bias)

            # ---- V projection (k-inner through pvA/pvB) ----
            def v_proj():
                for tt in range(NTT):
                    ps = pv_tile(tt % 2, f"v{tt}")
                    psv = ps[:, 0:G]
                    nc.tensor.matmul(out=psv, lhsT=ones_row[0:1, 0:128],
                                     rhs=bv_sb[0:1, :], start=True, stop=False)
                    for k in range(NKT):
                        nc.tensor.matmul(
                            out=psv,
                            lhsT=xt[k][:, tt * 128:(tt + 1) * 128],
                            rhs=wv_sb[k][:, :],
                            start=False, stop=(k == NKT - 1))
                    pr = psv.rearrange("p (b h x) -> p h b x", b=2, h=2)
                    nc.vector.tensor_copy(out=va_view[:, tt, :, 0:64],
                                          in_=pr[:, 0, :, :])
                    nc.vector.tensor_copy(out=va_view[:, tt, :, 128:192],
                                          in_=pr[:, 1, :, :])

            # ---- attention stages ----
            # Units: u in [0,32) -> key tile u//2, head u%2.  Score tiles
            # alternate ab3 (3 units) / ab2 (2 units); exp per tile.
            ab_idx = [3]   # global alternation counter (warm=1, kq_a=2)

            def attn_scores(c, pair):
                pa_locs = []
                u = 0
                while u < NU:
                    ab, slots = ab_tile(ab_idx[0], f"s{c}_{pair}_{u}")
                    ab_idx[0] += 1
                    nslots = min(slots, NU - u)
                    width = 512 * nslots
                    for s in range(nslots):
                        g, h = divmod(u, 2)
                        nc.tensor.matmul(
                            out=ab[:, s * 512:(s + 1) * 512],
                            lhsT=kt[pair][h * 64:(h + 1) * 64,
                                          g * 128:(g + 1) * 128],
                            rhs=qt[pair][h * 64:(h + 1) * 64,
                                         c * 512:(c + 1) * 512],
                            start=True, stop=True)
                        u += 1
                    pa = dyn.tile([128, 1536], fc, tag="pa", bufs=PA_BUFS,
                                  name=f"p{c}_{pair}_{u}")
                    nc.scalar.activation(out=pa[:, 0:width], in_=ab[:, 0:width],
                                         func=EXP, scale=0.125)
                    for s in range(nslots):
                        pa_locs.append((pa, s * 512))
                return pa_locs

            def attn_pv(c, pair, pa_locs):
                """PV accumulation; PSUM is evicted to SBUF immediately (two
                cheap copies) so the banks free in ~1.4us instead of being
                held through the ~8us normalize chain; normalization then
                runs from SBUF entirely off the PSUM dependency chain."""
                pv = [pv_tile(h, f"pv{h}_{c}_{pair}") for h in range(2)]
                for g in range(NTT):
                    off = g * VROW + pair * 192
                    p0, o0 = pa_locs[2 * g]
                    p1, o1 = pa_locs[2 * g + 1]
                    nc.tensor.matmul(
                        out=pv[0], lhsT=va[:, off:off + 128],
                        rhs=p0[:, o0:o0 + 512],
                        start=(g == 0), stop=(g == NTT - 1))
                    nc.tensor.matmul(
                        out=pv[1], lhsT=va[:, off + 64:off + 192],
                        rhs=p1[:, o1:o1 + 512],
                        start=(g == 0), stop=(g == NTT - 1))
                # Eviction does the partition alignment (PSUM reads can land
                # on any SBUF base partition): ovs = [O_h0 ; O_h1], dns =
                # [denom_h0 ; denom_h1] -> one reciprocal + one multiply for
                # both heads.
                ovs = dyn.tile([128, 512], f32, tag="ovs", bufs=2,
                               name=f"ovs_{c}_{pair}")
                dns = dyn.tile([128, 512], f32, tag="dns", bufs=2,
                               name=f"dns_{c}_{pair}")
                nc.vector.tensor_copy(out=ovs[0:64, :], in_=pv[0][0:64, :])
                nc.vector.tensor_copy(out=dns[0:64, :], in_=pv[0][64:128, :])
                nc.vector.tensor_copy(out=ovs[64:128, :], in_=pv[1][64:128, :])
                nc.vector.tensor_copy(out=dns[64:128, :], in_=pv[1][0:64, :])
                on = dyn.tile([128, 512], fc, tag=f"on{pair}", bufs=3,
                              name=f"on{pair}_{c}")
                rc = dyn.tile([128, 512], f32, tag="rc", bufs=2,
                              name=f"rc{c}_{pair}")
                nc.vector.reciprocal(out=rc, in_=dns)
                nc.vector.tensor_mul(on, ovs, rc)
                return on

            def outproj(c, ons):
                for mt in range(4):
                    osb = dyn.tile([128, 1024], fc, tag="osb", bufs=3,
                                   name=f"osb{c}_{mt}")
                    for n2 in range(2):
                        ps = op_tile(f"op{c}_{mt}_{n2}")
                        for pair in range(2):
                            nc.tensor.matmul(
                                out=ps,
                                lhsT=ons[pair][:, mt * 128:(mt + 1) * 128],
                                rhs=wo_sb[pair][:, n2 * 512:(n2 + 1) * 512],
                                start=(pair == 0), stop=(pair == 1))
                        nc.vector.tensor_copy(
                            out=osb[:, n2 * 512:(n2 + 1) * 512], in_=ps)
                    nc.sync.dma_start(
                        out=out[c * 512 + mt * 128:c * 512 + (mt + 1) * 128, :],
                        in_=osb)

            def tail_finish(c, pa_locs, on0):
                """Last (chunk, pair): PV + latency-optimized normalize +
                output projection.  Only the denominators are copied out of
                PSUM (muls read PSUM directly - mixed-space tensor_tensor is
                allowed); the reciprocal is split in halves with the output
                projection interleaved; outproj accumulates in the freed
                score banks (fully pipelined) and evicts on the idle
                ScalarE; junk fillers keep the PE warm through the DVE
                normalize window."""
                pv = [pv_tile(h, f"pvT{h}") for h in range(2)]
                for g in range(NTT):
                    off = g * VROW + 192
                    p0, o0 = pa_locs[2 * g]
                    p1, o1 = pa_locs[2 * g + 1]
                    nc.tensor.matmul(
                        out=pv[0], lhsT=va[:, off:off + 128],
                        rhs=p0[:, o0:o0 + 512],
                        start=(g == 0), stop=(g == NTT - 1))
                    nc.tensor.matmul(
                        out=pv[1], lhsT=va[:, off + 64:off + 192],
                        rhs=p1[:, o1:o1 + 512],
                        start=(g == 0), stop=(g == NTT - 1))
                wt, _ = ab_tile(ab_idx[0], "wtail")
                ab_idx[0] += 1
                for _ in range(18):
                    nc.tensor.matmul(out=wt[:, 0:512], lhsT=junk[:, 0:128],
                                     rhs=junk[:, 0:512], start=True, stop=True)
                dns = dyn.tile([128, 512], f32, tag="dns", bufs=2,
                               name="dnsT")
                nc.vector.tensor_copy(out=dns[0:64, :], in_=pv[0][64:128, :])
                nc.vector.tensor_copy(out=dns[64:128, :], in_=pv[1][0:64, :])
                on1 = dyn.tile([128, 512], fc, tag="on1", bufs=3, name="onT")
                rc = dyn.tile([128, 512], f32, tag="rc", bufs=2, name="rcT")
                ons = (on0, on1)
                slots = []   # (tile, offset) outproj accumulators in ab banks
                for half in range(2):
                    hs = slice(half * 256, (half + 1) * 256)
                    nc.vector.reciprocal(out=rc[:, hs], in_=dns[:, hs])
                    nc.vector.tensor_mul(on1[0:64, hs], pv[0][0:64, hs],
                                         rc[0:64, hs])
                    nc.vector.tensor_mul(on1[64:128, hs], pv[1][64:128, hs],
                                         rc[64:128, hs])
                    for mt in (2 * half, 2 * half + 1):
                        osb = dyn.tile([128, 1024], fc, tag="osb", bufs=3,
                                       name=f"osbT_{mt}")
                        for n2 in range(2):
                            if not slots:
                                t, ns = ab_tile(ab_idx[0], f"opT{mt}{n2}")
                                ab_idx[0] += 1
                                slots = [t[:, s * 512:(s + 1) * 512]
                                         for s in range(ns)]
                            ps = slots.pop(0)
                            for pair in range(2):
                                nc.tensor.matmul(
                                    out=ps,
                                    lhsT=ons[pair][:, mt * 128:(mt + 1) * 128],
                                    rhs=wo_sb[pair][:, n2 * 512:(n2 + 1) * 512],
                                    start=(pair == 0), stop=(pair == 1))
                            nc.scalar.activation(
                                out=osb[:, n2 * 512:(n2 + 1) * 512], in_=ps,
                                func=COPY)
                        nc.sync.dma_start(
                            out=out[c * 512 + mt * 128:
                                    c * 512 + (mt + 1) * 128, :],
                            in_=osb)

            # ---- schedule ----
            # Score streams lead in priority (ScalarE never starves); each
            # pair's PV+normalize follows its own stream; outproj trails by
            # TWO streams so its (mis-cost-modeled) normalize dependency
            # can never stall the tensor queue.
            sc00 = attn_scores(0, 0)       # exp starts here (~21us)
            kq_proj(1, wk_sb, kt[1], None)          # under sc00's exp stream
            kq_proj(1, wq_sb, qt[1], bq_sb[:, 1:2])
            sc01 = attn_scores(0, 1)
            v_proj()                                # under sc01's exp stream
            sc10 = attn_scores(1, 0)
            on00 = attn_pv(0, 0, sc00)              # bursts after V is done
            on01 = attn_pv(0, 1, sc01)
            on10 = attn_pv(1, 0, sc10)
            sc11 = attn_scores(1, 1)
            on11 = attn_pv(1, 1, sc11)
            q0_rest()
            sc20 = attn_scores(2, 0)
            on20 = attn_pv(2, 0, sc20)
            outproj(0, (on00, on01))
            sc21 = attn_scores(2, 1)
            on21 = attn_pv(2, 1, sc21)
            outproj(1, (on10, on11))
            sc30 = attn_scores(3, 0)
            on30 = attn_pv(3, 0, sc30)
            outproj(2, (on20, on21))
            sc31 = attn_scores(3, 1)
            tail_finish(3, sc31, on30)

    if split_waits:
        _split_multi_waits(nc)
    return nc


COMPUTE_DT = "float16"   # matmul operand dtype; PSUM accumulation is fp32


def _get_nc(split_waits=True, compute_dt=COMPUTE_DT):
    key = ("nc", split_waits, compute_dt)
    if key not in _CACHE:
        _CACHE[key] = _build(split_waits, compute_dt)
    return _CACHE[key]


def _np_dt():
    return {"float16": np.float16, "bfloat16": None,
            "float32": np.float32}[COMPUTE_DT]


def make_in_maps(x, Wq, bq, Wk, bk, Wv, bv, Wo):
    # bk is intentionally unused: softmax is exactly invariant to it.
    dt = _np_dt()
    in_maps = []
    for core in range(8):
        b, g = divmod(core, 4)
        gs = slice(g * G, (g + 1) * G)
        in_maps.append({
            "xT": np.ascontiguousarray(x[b].T).astype(dt),
            "wqT": np.ascontiguousarray(Wq[gs, :].T).astype(dt),
            "wkT": np.ascontiguousarray(Wk[gs, :].T).astype(dt),
            "wvT": np.ascontiguousarray(Wv[gs, :].T).astype(dt),
            "woT": np.ascontiguousarray(Wo[:, gs].T).astype(dt),
            "bqT": np.ascontiguousarray(bq[gs].reshape(G, 1)).astype(np.float32),
            "bv": np.ascontiguousarray(bv[gs].reshape(1, G)).astype(dt),
        })
    return in_maps


def kernel(x, Wq, bq, Wk, bk, Wv, bv, Wo, bo):
    from concourse.bass_utils import run_bass_kernel_spmd

    x = np.asarray(x, dtype=np.float32)
    Wq = np.asarray(Wq, dtype=np.float32)
    Wk = np.asarray(Wk, dtype=np.float32)
    Wv = np.asarray(Wv, dtype=np.float32)
    Wo = np.asarray(Wo, dtype=np.float32)
    bq = np.asarray(bq, dtype=np.float32)
    bv = np.asarray(bv, dtype=np.float32)
    bo = np.asarray(bo, dtype=np.float32)

    nc = _get_nc()
    in_maps = make_in_maps(x, Wq, bq, Wk, None, Wv, bv, Wo)

    res = run_bass_kernel_spmd(nc, in_maps, core_ids=list(range(8)))
    outp = np.tile(bo[None, None, :], (2, T, 1)).astype(np.float32)
    for core in range(8):
        b = core // 4
        outp[b] += res.results[core]["out"].astype(np.float32)
    return outp


# revision 35
# speedup vs baseline: 1.0286x; 1.0242x over previous
"""Multi-head attention Trainium2 Bass kernel (v6 — overlap-optimized).

Problem: B=2, T=2048, D=1024, H=16 heads, dk=64 (fp32).
  out = softmax((x@Wq.T+bq)(x@Wk.T+bk).T / 8) (x@Wv.T+bv) @ Wo.T + bo

Sharding (8 cores): data-parallel over B (2) x tensor-parallel over 4
head-groups of 4 heads.  Core (b, g) computes, for batch b and heads
[4g, 4g+4):  Q/K/V projections (column-sliced Wq/Wk/Wv), attention, and
the row-sliced Wo projection, producing a partial (2048, 1024) fp16
output.  Host sums the 4 group partials per batch and adds bo.

Design (v6):
  - bk dropped entirely (softmax exactly invariant to it); bq folded
    into the Q eviction via DVE tensor_scalar (per-partition scalar).
  - ScalarE softmax exp is the end-to-end pacer (16.8M exps/core at
    1 elem/cycle/lane @1.2GHz).  Score tiles are [128,1536] (3 PSUM
    banks, 1.5 key tiles x 2 heads) so each exp ACTIVATE amortizes its
    ~300ns overhead over 1536 columns: ~132us total.  ScalarE does exp
    (+ the K eviction copies before exp starts, + the final chunk's
    output evictions after it ends); everything else is on the DVE.
  - Startup: input DMAs split across both HWDGE queues in consumption
    order; 12 junk warmup matmuls (uninitialized-SBUF operands, no DMA
    dependency) right after the ~7us runtime prologue heat the PE HAM
    to 2.4GHz, and more junk fillers are interleaved into the K/Q
    pair-0 k-outer projection loop so the PE never idles (and never
    re-throttles) while the x stream trickles in.  K+Q pair-0 run
    k-outer simultaneously across all 8 PSUM banks in x-arrival order;
    the first softmax exp fires ~21us in.
  - PSUM budget (8 banks): ab pool 2x[128,1536] double-buffered score
    tiles + pvA/pvB [128,256] bufs=2 (2 banks total) shared by the
    pair-1/V projection accumulators, PV accumulation (per-head
    half-chunks of 256 queries), and the output projection quarters.
  - The scheduler's cost model does not know DVE RECIPROCAL is ~7x
    slower than a copy, so any tensor instruction scheduled close
    behind a reciprocal stalls the in-order tensor queue at runtime.
    Hence: normalization is chunked ([64,128] reciprocal + multiply,
    short chain links), PV double-buffering keeps the next pair's PV
    off the normalize chain, and outproj(c) is emitted TWO exp-streams
    later so its normalize dependency is long resolved by the time it
    reaches the tensor queue head.
  - Scores per key tile are an adjacent pair of K=64 matmuls on PE row
    strips 0-63/64-127 (auto tile_position) which the HW row-packs and
    runs concurrently (verified ~3ns apart in traces).
  - V_aug [V_he | ones64 | V_ho] blocks make the PV matmul emit the
    softmax denominator replicated across 64 partitions.
  - No max-subtraction in softmax (|S|/8 < ~3, fp32-exact regime).
"""

import numpy as np

D = 1024          # d_model
T = 2048          # sequence length
G = 256           # features per head-group (4 heads * 64)
DK = 64
NKT = D // 128    # 8 contraction tiles for projections
NTT = T // 128    # 16 T tiles (key tiles)
NCH = T // 512    # 4 query chunks of 512
VROW = 2 * 192    # V_aug row: 2 blocks of [V_he | ones64 | V_ho]
NU = 2 * NTT      # 32 (key-tile, head) units per (chunk, pair)
NAB = (NU + 2) // 3   # 11 score tiles per (chunk, pair)
PA_BUFS = 32      # exp tiles in flight

_CACHE = {}


def _split_multi_waits(nc):
    """walrus's TRN2 codegen rejects >1 sync-wait on datapath instruction
    structs (e.g. the fp32 self-loading matmul's LDWEIGHTS part, tensor-
    scalar).  Hoist every wait of a multi-wait datapath instruction onto
    single-wait NoOps just before it on the same engine queue - semantically
    identical (engine executes in order) and each NoOp carries one wait."""
    import concourse.mybir as mybir

    keep = ("InstEventSemaphore", "InstUnconditionalBranch",
            "InstCall", "InstBranchHint", "InstHalt", "InstNoOp",
            "InstAllEngineBarrier", "InstCompareAndBranch")
    nid = [0]
    for f in nc.m.functions:
        for bb in f.blocks:
            new = []
            for ins in bb.instructions:
                si = ins.sync_info
                waits = list(si.on_wait) if si and si.on_wait else []
                if len(waits) >= 2 and type(ins).__name__ not in keep:
                    for w in waits:
                        nid[0] += 1
                        nop = mybir.InstNoOp(name=f"{ins.name}-wsplit{nid[0]}",
                                             ins=[], outs=[])
                        nop.engine = ins.engine
                        nop.sync_info = mybir.SyncInfo(on_wait=[w], on_update=[])
                        new.append(nop)
                    ins.sync_info = mybir.SyncInfo(
                        on_wait=[], on_update=list(si.on_update or []))
                new.append(ins)
            bb.instructions = new


def _build(split_waits=True, compute_dt="float16"):
    import concourse.bass as bass
    import concourse.mybir as mybir
    import concourse.tile as tile

    f32 = mybir.dt.float32
    fc = getattr(mybir.dt, compute_dt)
    nc = bass.Bass()

    xT = nc.dram_tensor("xT", [D, T], fc, kind="ExternalInput")
    wqT = nc.dram_tensor("wqT", [D, G], fc, kind="ExternalInput")
    wkT = nc.dram_tensor("wkT", [D, G], fc, kind="ExternalInput")
    wvT = nc.dram_tensor("wvT", [D, G], fc, kind="ExternalInput")
    woT = nc.dram_tensor("woT", [G, D], fc, kind="ExternalInput")
    bqT = nc.dram_tensor("bqT", [G, 1], f32, kind="ExternalInput")
    bv = nc.dram_tensor("bv", [1, G], fc, kind="ExternalInput")
    out = nc.dram_tensor("out", [T, D], fc, kind="ExternalOutput")

    EXP = mybir.ActivationFunctionType.Exp
    COPY = mybir.ActivationFunctionType.Copy

    with tile.TileContext(nc) as tc:
        with tc.tile_pool(name="sb", bufs=1) as sb, \
             tc.tile_pool(name="dyn", bufs=2) as dyn, \
             tc.tile_pool(name="ps_ab", bufs=1, space="PSUM") as ps_ab, \
             tc.tile_pool(name="ps_w", bufs=1, space="PSUM") as ps_w:
            # PSUM (8 banks, bank-granular): ab3 [128,1536] + ab2 [128,1024]
            # alternating score tiles (5 banks, behaves like one
            # double-buffered tag), pvA [128,512] bufs=2 + pvB bufs=1.

            def ab_tile(i, name):
                tag = ("ab3", "ab2")[i % 2]
                shape = [128, 1536] if tag == "ab3" else [128, 1024]
                return ps_ab.tile(shape, f32, tag=tag, name=name), shape[1] // 512

            def pv_tile(h, name):
                return ps_w.tile([128, 512], f32, tag=("pvA", "pvB")[h],
                                 bufs=1, name=name)

            def op_tile(name):
                return ps_w.tile([128, 512], f32, tag="op", bufs=1, name=name)

            # ---- constants + exp table preload ----
            ones_row = sb.tile([1, G], fc, tag="ones", name="ones_row")
            nc.vector.memset(ones_row, 1.0)
            dummy = sb.tile([1, 1], fc, tag="dummy", name="dummy")
            nc.scalar.activation(out=dummy, in_=ones_row[0:1, 0:1], func=EXP)

            # ---- input DMAs, split across both HWDGE queues ----
            xt = [None] * NKT
            wk_sb, wq_sb, wv_sb = [], [], []
            # sync queue: wk+wq (small, needed first), then x0..x2
            for k in range(NKT):
                t = sb.tile([128, G], fc, tag=f"wk{k}", name=f"wk{k}")
                nc.sync.dma_start(out=t, in_=wkT[k * 128:(k + 1) * 128, :])
                wk_sb.append(t)
                t = sb.tile([128, G], fc, tag=f"wq{k}", name=f"wq{k}")
                nc.sync.dma_start(out=t, in_=wqT[k * 128:(k + 1) * 128, :])
                wq_sb.append(t)
            for k in range(3):
                t = sb.tile([128, T], fc, tag=f"xt{k}", name=f"xt{k}")
                nc.sync.dma_start(out=t, in_=xT[k * 128:(k + 1) * 128, :])
                xt[k] = t
            # scalar queue: x4..x6, then biases, wv, wo
            for k in (4, 5, 6):
                t = sb.tile([128, T], fc, tag=f"xt{k}", name=f"xt{k}")
                nc.scalar.dma_start(out=t, in_=xT[k * 128:(k + 1) * 128, :])
                xt[k] = t
            # gpsimd SWDGE queue: x3, x7 (third parallel DMA path)
            for k in (3, 7):
                t = sb.tile([128, T], fc, tag=f"xt{k}", name=f"xt{k}")
                nc.gpsimd.dma_start(out=t, in_=xT[k * 128:(k + 1) * 128, :])
                xt[k] = t
            bq_sb = sb.tile([128, 2], f32, tag="bq", name="bq_sb")
            nc.scalar.dma_start(out=bq_sb[:, 0:1], in_=bqT[0:128, :])
            nc.scalar.dma_start(out=bq_sb[:, 1:2], in_=bqT[128:256, :])
            bv_sb = sb.tile([1, G], fc, tag="bv", name="bv_sb")
            nc.scalar.dma_start(out=bv_sb, in_=bv[:, :])
            for k in range(NKT):
                t = sb.tile([128, G], fc, tag=f"wv{k}", name=f"wv{k}")
                nc.scalar.dma_start(out=t, in_=wvT[k * 128:(k + 1) * 128, :])
                wv_sb.append(t)
            wo_sb = []
            for p2 in range(2):
                t = sb.tile([128, D], fc, tag=f"wo{p2}", name=f"wo{p2}")
                nc.scalar.dma_start(out=t, in_=woT[p2 * 128:(p2 + 1) * 128, :])
                wo_sb.append(t)

            # V_aug storage: per T-tile 2 blocks of [V_he|ones64|V_ho]
            va = sb.tile([128, NTT * VROW], fc, tag="va", name="va")
            va_view = va.rearrange("p (t b x) -> p t b x", t=NTT, b=2)
            for b2 in range(2):
                nc.vector.memset(va_view[:, :, b2, 64:128], 1.0)

            qt = [sb.tile([128, T], fc, tag=f"qt{p}", name=f"qt{p}")
                  for p in range(2)]
            kt = [sb.tile([128, T], fc, tag=f"kt{p}", name=f"kt{p}")
                  for p in range(2)]

            # ---- HAM warmup: junk matmuls on an uninitialized SBUF tile
            # (no DMA dependency) heat the PE right after the prologue.
            junk = sb.tile([128, 512], fc, tag="junk", name="junk")
            nc.vector.memset(junk, 1.0)
            warm, _ = ab_tile(1, "warm")   # ab2 bank; released at phase-A end

            def filler(n):
                for _ in range(n):
                    nc.tensor.matmul(out=warm[:, 0:512], lhsT=junk[:, 0:128],
                                     rhs=junk[:, :], start=True, stop=True)

            filler(12)

            # ---- K pair-0 (all chunks) + Q pair-0 (chunks 0,1): k-outer in
            # x-tile arrival order, junk fillers interleaved so the PE never
            # re-throttles while the x stream trickles in.  scores(0,0) only
            # needs Q chunk 0, so Q0 c2/c3 are computed later as stream
            # filler.
            kq_a, _ = ab_tile(2, "kq_a")    # ab3: K0 c0,c1,c2
            pv_k3 = op_tile("kq_k3")        # op bank: K0 c3
            pv_q0 = pv_tile(0, "kq_q0")     # pvA: Q0 c0
            pv_q1 = pv_tile(1, "kq_q1")     # pvB: Q0 c1
            kacc = [kq_a[:, 0:512], kq_a[:, 512:1024], kq_a[:, 1024:1536],
                    pv_k3]
            qacc = [pv_q0, pv_q1]
            korder = [4, 0, 5, 1, 6, 2, 3, 7]   # x-tile arrival order
            for i, k in enumerate(korder):
                st, sp = (i == 0), (i == len(korder) - 1)
                for c in range(NCH):
                    nc.tensor.matmul(out=kacc[c], lhsT=wk_sb[k][:, 0:128],
                                     rhs=xt[k][:, c * 512:(c + 1) * 512],
                                     start=st, stop=sp)
                for c in range(2):
                    nc.tensor.matmul(out=qacc[c], lhsT=wq_sb[k][:, 0:128],
                                     rhs=xt[k][:, c * 512:(c + 1) * 512],
                                     start=st, stop=sp)
                filler(2)
            # evictions: kt on ScalarE (idle until exp starts), qt on DVE
            for c in range(NCH):
                nc.scalar.activation(out=kt[0][:, c * 512:(c + 1) * 512],
                                     in_=kacc[c], func=COPY)
            for c in range(2):
                nc.vector.tensor_scalar_add(qt[0][:, c * 512:(c + 1) * 512],
                                            qacc[c], bq_sb[:, 0:1])

            def q0_rest():
                for cc in (2, 3):
                    acc = pv_tile(cc - 2, f"q0r{cc}")
                    for k in range(NKT):
                        nc.tensor.matmul(
                            out=acc, lhsT=wq_sb[k][:, 0:128],
                            rhs=xt[k][:, cc * 512:(cc + 1) * 512],
                            start=(k == 0), stop=(k == NKT - 1))
                    nc.vector.tensor_scalar_add(
                        qt[0][:, cc * 512:(cc + 1) * 512], acc, bq_sb[:, 0:1])

            # ---- K/Q pair-1 (k-inner, 2 chunks at a time; x resident) ----
            def kq_proj(p2, w_sb, dst, bias):
                for cc in (0, 2):
                    accs = [pv_tile(h, f"kq{p2}_{cc}_{h}") for h in range(2)]
                    for k in range(NKT):
                        for j in range(2):
                            c = cc + j
                            nc.tensor.matmul(
                                out=accs[j],
                                lhsT=w_sb[k][:, p2 * 128:(p2 + 1) * 128],
                                rhs=xt[k][:, c * 512:(c + 1) * 512],
                                start=(k == 0), stop=(k == NKT - 1))
                    for j in range(2):
                        c = cc + j
                        if bias is None:
                            nc.vector.tensor_copy(
                                out=dst[:, c * 512:(c + 1) * 512], in_=accs[j])
                        else:
                            nc.vector.tensor_scalar_add(
                                dst[:, c * 512:(c + 1) * 512], accs[j], bias)

            # ---- V projection (k-inner through pvA/pvB) ----
            def v_proj():
                for tt in range(NTT):
                    ps = pv_tile(tt % 2, f"v{tt}")
                    psv = ps[:, 0:G]
                    nc.tensor.matmul(out=psv, lhsT=ones_row[0:1, 0:128],
                                     rhs=bv_sb[0:1, :], start=True, stop=False)
                    for k in range(NKT):
                        nc.tensor.matmul(
                            out=psv,
                            lhsT=xt[k][:, tt * 128:(tt + 1) * 128],
                            rhs=wv_sb[k][:, :],
                            start=False, stop=(k == NKT - 1))
                    pr = psv.rearrange("p (b h x) -> p h b x", b=2, h=2)
                    nc.vector.tensor_copy(out=va_view[:, tt, :, 0:64],
                                          in_=pr[:, 0, :, :])
                    nc.vector.tensor_copy(out=va_view[:, tt, :, 128:192],
                                          in_=pr[:, 1, :, :])

            # ---- attention stages ----
            # Units: u in [0,32) -> key tile u//2, head u%2.  Score tiles
            # alternate ab3 (3 units) / ab2 (2 units); exp per tile.
            ab_idx = [3]   # global alternation counter (warm=1, kq_a=2)

            def attn_scores(c, pair):
                pa_locs = []
                u = 0
                while u < NU:
                    ab, slots = ab_tile(ab_idx[0], f"s{c}_{pair}_{u}")
                    ab_idx[0] += 1
                    nslots = min(slots, NU - u)
                    width = 512 * nslots
                    for s in range(nslots):
                        g, h = divmod(u, 2)
                        nc.tensor.matmul(
                            out=ab[:, s * 512:(s + 1) * 512],
                            lhsT=kt[pair][h * 64:(h + 1) * 64,
                                          g * 128:(g + 1) * 128],
                            rhs=qt[pair][h * 64:(h + 1) * 64,
                                         c * 512:(c + 1) * 512],
                            start=True, stop=True)
                        u += 1
                    pa = dyn.tile([128, 1536], fc, tag="pa", bufs=PA_BUFS,
                                  name=f"p{c}_{pair}_{u}")
                    nc.scalar.activation(out=pa[:, 0:width], in_=ab[:, 0:width],
                                         func=EXP, scale=0.125)
                    for s in range(nslots):
                        pa_locs.append((pa, s * 512))
                return pa_locs

            def attn_pv(c, pair, pa_locs):
                """PV accumulation; PSUM is evicted to SBUF immediately (two
                cheap copies) so the banks free in ~1.4us instead of being
                held through the ~8us normalize chain; normalization then
                runs from SBUF entirely off the PSUM dependency chain."""
                pv = [pv_tile(h, f"pv{h}_{c}_{pair}") for h in range(2)]
                for g in range(NTT):
                    off = g * VROW + pair * 192
                    p0, o0 = pa_locs[2 * g]
                    p1, o1 = pa_locs[2 * g + 1]
                    nc.tensor.matmul(
                        out=pv[0], lhsT=va[:, off:off + 128],
                        rhs=p0[:, o0:o0 + 512],
                        start=(g == 0), stop=(g == NTT - 1))
                    nc.tensor.matmul(
                        out=pv[1], lhsT=va[:, off + 64:off + 192],
                        rhs=p1[:, o1:o1 + 512],
                        start=(g == 0), stop=(g == NTT - 1))
                # Eviction does the partition alignment (PSUM reads can land
                # on any SBUF base partition): ovs = [O_h0 ; O_h1], dns =
                # [denom_h0 ; denom_h1] -> one reciprocal + one multiply for
                # both heads.
                ovs = dyn.tile([128, 512], f32, tag="ovs", bufs=2,
                               name=f"ovs_{c}_{pair}")
                dns = dyn.tile([128, 512], f32, tag="dns", bufs=2,
                               name=f"dns_{c}_{pair}")
                nc.vector.tensor_copy(out=ovs[0:64, :], in_=pv[0][0:64, :])
                nc.vector.tensor_copy(out=dns[0:64, :], in_=pv[0][64:128, :])
                nc.vector.tensor_copy(out=ovs[64:128, :], in_=pv[1][64:128, :])
                nc.vector.tensor_copy(out=dns[64:128, :], in_=pv[1][0:64, :])
                on = dyn.tile([128, 512], fc, tag=f"on{pair}", bufs=3,
                              name=f"on{pair}_{c}")
                rc = dyn.tile([128, 512], f32, tag="rc", bufs=2,
                              name=f"rc{c}_{pair}")
                nc.vector.reciprocal(out=rc, in_=dns)
                nc.vector.tensor_mul(on, ovs, rc)
                return on

            def outproj(c, ons):
                for mt in range(4):
                    osb = dyn.tile([128, 1024], fc, tag="osb", bufs=3,
                                   name=f"osb{c}_{mt}")
                    for n2 in range(2):
                        ps = op_tile(f"op{c}_{mt}_{n2}")
                        for pair in range(2):
                            nc.tensor.matmul(
                                out=ps,
                                lhsT=ons[pair][:, mt * 128:(mt + 1) * 128],
                                rhs=wo_sb[pair][:, n2 * 512:(n2 + 1) * 512],
                                start=(pair == 0), stop=(pair == 1))
                        nc.vector.tensor_copy(
                            out=osb[:, n2 * 512:(n2 + 1) * 512], in_=ps)
                    nc.sync.dma_start(
                        out=out[c * 512 + mt * 128:c * 512 + (mt + 1) * 128, :],
                        in_=osb)

            def tail_finish(c, pa_locs, on0):
                """Last (chunk, pair): PV + latency-optimized normalize +
                output projection.  Only the denominators are copied out of
                PSUM (muls read PSUM directly - mixed-space tensor_tensor is
                allowed); the reciprocal is split in halves with the output
                projection interleaved; outproj accumulates in the freed
                score banks (fully pipelined) and evicts on the idle
                ScalarE; junk fillers keep the PE warm through the DVE
                normalize window."""
                pv = [pv_tile(h, f"pvT{h}") for h in range(2)]
                for g in range(NTT):
                    off = g * VROW + 192
                    p0, o0 = pa_locs[2 * g]
                    p1, o1 = pa_locs[2 * g + 1]
                    nc.tensor.matmul(
                        out=pv[0], lhsT=va[:, off:off + 128],
                        rhs=p0[:, o0:o0 + 512],
                        start=(g == 0), stop=(g == NTT - 1))
                    nc.tensor.matmul(
                        out=pv[1], lhsT=va[:, off + 64:off + 192],
                        rhs=p1[:, o1:o1 + 512],
                        start=(g == 0), stop=(g == NTT - 1))
                wt, _ = ab_tile(ab_idx[0], "wtail")
                ab_idx[0] += 1
                for _ in range(18):
                    nc.tensor.matmul(out=wt[:, 0:512], lhsT=junk[:, 0:128],
                                     rhs=junk[:, 0:512], start=True, stop=True)
                dns = dyn.tile([128, 512], f32, tag="dns", bufs=2,
                               name="dnsT")
                nc.vector.tensor_copy(out=dns[0:64, :], in_=pv[0][64:128, :])
                nc.vector.tensor_copy(out=dns[64:128, :], in_=pv[1][0:64, :])
                on1 = dyn.tile([128, 512], fc, tag="on1", bufs=3, name="onT")
                rc = dyn.tile([128, 512], f32, tag="rc", bufs=2, name="rcT")
                ons = (on0, on1)
                slots = []   # (tile, offset) outproj accumulators in ab banks
                for half in range(2):
                    hs = slice(half * 256, (half + 1) * 256)
                    nc.vector.reciprocal(out=rc[:, hs], in_=dns[:, hs])
                    nc.vector.tensor_mul(on1[0:64, hs], pv[0][0:64, hs],
                                         rc[0:64, hs])
                    nc.vector.tensor_mul(on1[64:128, hs], pv[1][64:128, hs],
                                         rc[64:128, hs])
                    for mt in (2 * half, 2 * half + 1):
                        osb = dyn.tile([128, 1024], fc, tag="osb", bufs=3,
                                       name=f"osbT_{mt}")
                        for n2 in range(2):
                            if not slots:
                                t, ns = ab_tile(ab_idx[0], f"opT{mt}{n2}")
                                ab_idx[0] += 1
                                slots = [t[:, s * 512:(s + 1) * 512]
                                         for s in range(ns)]
                            ps = slots.pop(0)
                            for pair in range(2):
                                nc.tensor.matmul(
                                    out=ps,
                                    lhsT=ons[pair][:, mt * 128:(mt + 1) * 128],
                                    rhs=wo_sb[pair][:, n2 * 512:(n2 + 1) * 512],
                                    start=(pair == 0), stop=(pair == 1))
                            nc.scalar.activation(
                                out=osb[:, n2 * 512:(n2 + 1) * 512], in_=ps,
                                func=COPY)
                        nc.sync.dma_start(
                            out=out[c * 512 + mt * 128:
                                    c * 512 + (mt + 1) * 128, :],
                            in_=osb)

            # ---- schedule ----
            # Score streams lead in priority (ScalarE never starves); each
            # pair's PV+normalize follows its own stream; outproj trails by
            # TWO streams so its (mis-cost-modeled) normalize dependency
            # can never stall the tensor queue.
            sc00 = attn_scores(0, 0)       # exp starts here (~21us)
            kq_proj(1, wk_sb, kt[1], None)          # under sc00's exp stream
            q0_rest()
            kq_proj(1, wq_sb, qt[1], bq_sb[:, 1:2])
            sc01 = attn_scores(0, 1)
            v_proj()                                # under sc01's exp stream
            sc10 = attn_scores(1, 0)
            on00 = attn_pv(0, 0, sc00)              # bursts after V is done
            on01 = attn_pv(0, 1, sc01)
            on10 = attn_pv(1, 0, sc10)
            sc11 = attn_scores(1, 1)
            on11 = attn_pv(1, 1, sc11)
            sc20 = attn_scores(2, 0)
            on20 = attn_pv(2, 0, sc20)
            outproj(0, (on00, on01))
            sc21 = attn_scores(2, 1)
            on21 = attn_pv(2, 1, sc21)
            outproj(1, (on10, on11))
            sc30 = attn_scores(3, 0)
            on30 = attn_pv(3, 0, sc30)
            outproj(2, (on20, on21))
            sc31 = attn_scores(3, 1)
            tail_finish(3, sc31, on30)

    if split_waits:
        _split_multi_waits(nc)
    return nc


COMPUTE_DT = "float16"   # matmul operand dtype; PSUM accumulation is fp32


def _get_nc(split_waits=True, compute_dt=COMPUTE_DT):
    key = ("nc", split_waits, compute_dt)
    if key not in _CACHE:
        _CACHE[key] = _build(split_waits, compute_dt)
    return _CACHE[key]


def _np_dt():
    return {"float16": np.float16, "bfloat16": None,
            "float32": np.float32}[COMPUTE_DT]


def make_in_maps(x, Wq, bq, Wk, bk, Wv, bv, Wo):
    # bk is intentionally unused: softmax is exactly invariant to it.
    dt = _np_dt()
    in_maps = []
    for core in range(8):
        b, g = divmod(core, 4)
        gs = slice(g * G, (g + 1) * G)
        in_maps.append({
            "xT": np.ascontiguousarray(x[b].T).astype(dt),
            "wqT": np.ascontiguousarray(Wq[gs, :].T).astype(dt),
            "wkT": np.ascontiguousarray(Wk[gs, :].T).astype(dt),
            "wvT": np.ascontiguousarray(Wv[gs, :].T).astype(dt),
            "woT": np.ascontiguousarray(Wo[:, gs].T).astype(dt),
            "bqT": np.ascontiguousarray(bq[gs].reshape(G, 1)).astype(np.float32),
            "bv": np.ascontiguousarray(bv[gs].reshape(1, G)).astype(dt),
        })
    return in_maps


def kernel(x, Wq, bq, Wk, bk, Wv, bv, Wo, bo):
    from concourse.bass_utils import run_bass_kernel_spmd

    x = np.asarray(x, dtype=np.float32)
    Wq = np.asarray(Wq, dtype=np.float32)
    Wk = np.asarray(Wk, dtype=np.float32)
    Wv = np.asarray(Wv, dtype=np.float32)
    Wo = np.asarray(Wo, dtype=np.float32)
    bq = np.asarray(bq, dtype=np.float32)
    bv = np.asarray(bv, dtype=np.float32)
    bo = np.asarray(bo, dtype=np.float32)

    nc = _get_nc()
    in_maps = make_in_maps(x, Wq, bq, Wk, None, Wv, bv, Wo)

    res = run_bass_kernel_spmd(nc, in_maps, core_ids=list(range(8)))
    outp = np.tile(bo[None, None, :], (2, T, 1)).astype(np.float32)
    for core in range(8):
        b = core // 4
        outp[b] += res.results[core]["out"].astype(np.float32)
    return outp
